# revision 4
# baseline (speedup 1.0000x reference)
"""Trainium2 Bass kernel for nn_BlipAttention_75007308857568.

Single-head BLIP attention: B=32, N=1024, C=768, fp32 in/out.
  qkv = x @ qkv_w + qkv_b ; q,k,v split
  scores = q @ k.T / sqrt(C) ; attn = softmax(scores)
  out = attn @ v
  y = (out.swapaxes(1,2).reshape(B,N,C)) @ proj_w + proj_b

Sharding: data-parallel over batch B across 8 NeuronCores (4 batches/core).

Per-core dataflow (all in "transposed domain" to keep matmul contraction
dims on SBUF partitions without redundant transposes):
  XT  = x[b].T                      (cast rows to fp16, 48 PE transposes)
  QT/KT = (Wq|Wk).T @ XT            (PE; q/k bias fused into the
                                     PSUM->SBUF copy as per-partition adds)
  Vg  = x[b] @ Wvg                  (PE; Wv columns pre-permuted so V comes
                                     out grouped by c mod 3 -- see below)
  scoresT[m,n] = KT.T@QT            (PE)
  expT = exp(scoresT/sqrt(C))       (ACT, PSUM->SBUF fp16)
  denom = ones128.T @ expT          (PE; denominator replicated on all
                                     128 partitions)
  expTn = expT * (S/denom)          (DVE 2-byte fast mode; S=128 keeps
                                     normalized weights in fp16 range)
  OT3 = (Vg.T @ expTn)/S + vb       (PE + fused copy; rows grouped c0=c%3,
                                     ordered by c//3, all in SBUF)
  PT_g = 48 PE transposes of OT3    (the swapaxes+reshape permutation,
                                     done entirely in SBUF: PT[jb*128+jp,i]
                                     = OT[c, nb*128+jp] with (c,nb) =
                                     divmod(6i+jb, 8); i=4t+r makes every
                                     128x128 transpose source a plain tile
                                     of the c0-grouped storage)
  y = PT_g.T @ proj_w + proj_b      (PE; output rows land permuted, the
                                     final DMA un-permutes with stride-4
                                     row writes)

All matmul operands are stored fp16 (PSUM accumulates fp32): same
1 cyc/row PE speed as float32r in steady state, but enables the
compiler's fast-weight-load path (fp32 stationary operands can't use it),
halves PE transpose cost, and removes the fp32 DRAM scratch round-trip
the old swapaxes implementation needed.
PSUM accumulation chains are paired into 2-bank [128,1024] tiles so each
PSUM->SBUF copy / exp covers two chains (half the instruction count).
"""

import math
import os

import numpy as np

import concourse.bacc as bacc
import concourse.bass as bass
import concourse.mybir as mybir
import concourse.tile as tile

from concourse.bass_utils import run_bass_kernel_spmd
from concourse.masks import make_identity

B, N, C = 32, 1024, 768
NCORES = 8
BPC = B // NCORES  # batches per core
CB = C // 128      # 6 channel blocks
NB = N // 128      # 8 sequence blocks
NH = 512           # n-half width (PSUM bank limit for fp32 accum)
SCALE = 1.0 / math.sqrt(C)
ASCALE = 128.0     # pre-normalized attention weights scaled by this

_CACHE = {}


def _build(mm_dtype: str):
    dt = mybir.dt
    MM = getattr(dt, mm_dtype)
    f32 = dt.float32
    mm_cast = MM != f32
    add, mult = mybir.AluOpType.add, mybir.AluOpType.mult

    nc = bacc.Bacc("TRN2", target_bir_lowering=False, debug=False)

    xs = nc.dram_tensor("xs", [BPC, N, C], f32, kind="ExternalInput")
    qkv_w = nc.dram_tensor("qkv_w", [C, 3 * C], f32, kind="ExternalInput")
    qkv_b = nc.dram_tensor("qkv_b", [3 * C], f32, kind="ExternalInput")
    proj_w = nc.dram_tensor("proj_w", [C, C], f32, kind="ExternalInput")
    proj_b = nc.dram_tensor("proj_b", [C], f32, kind="ExternalInput")
    y = nc.dram_tensor("y", [BPC, N, C], f32, kind="ExternalOutput")

    with tile.TileContext(nc) as tc:
        with (
            tc.tile_pool(name="consts", bufs=1) as consts,
            tc.tile_pool(name="wstage", bufs=1) as wstage,
            tc.tile_pool(name="xt", bufs=int(os.environ.get("BLIP_XT", "2"))) as pool_xt,
            tc.tile_pool(name="qt", bufs=1) as pool_qt,
            tc.tile_pool(name="kt", bufs=1) as pool_kt,
            tc.tile_pool(name="v", bufs=1) as pool_v,
            tc.tile_pool(name="expt", bufs=1) as pool_expt,
            tc.tile_pool(name="exptn", bufs=1) as pool_exptn,
            tc.tile_pool(name="ot3", bufs=1) as pool_ot3,
            tc.tile_pool(name="ptg", bufs=1) as pool_ptg,
            tc.tile_pool(name="row", bufs=int(os.environ.get("BLIP_ROW", "3"))) as pool_row,
            tc.tile_pool(name="rb", bufs=1) as pool_rb,
            tc.tile_pool(name="psmm", bufs=int(os.environ.get("BLIP_PSMM", "3")), space="PSUM") as psmm,
            tc.tile_pool(name="pst", bufs=int(os.environ.get("BLIP_PST", "2")), space="PSUM") as pst,
        ):
            # ---- constants / weights (loaded once) ----
            ident = consts.tile([128, 128], MM, tag="ident")
            make_identity(nc, ident)

            Wqk = consts.tile([128, CB, 2 * C], MM, tag="Wqk")
            Wvg = consts.tile([128, CB, C], MM, tag="Wvg")
            PW = consts.tile([128, CB, C], MM, tag="PW")

            # q/k bias as per-partition scalars: qkb[p, ob] = qkv_b[ob*128+p]
            qkb = consts.tile([128, 2 * CB], f32, tag="qkb")
            nc.sync.dma_start(
                qkb, qkv_b.ap()[0 : 2 * C].rearrange("(ob p) -> p ob", p=128)
            )
            # v bias, permuted to the (c0, T) grouping of OT3:
            # vbc[p, 2*c0+T] = qkv_b[2C + 3*(128T+p) + c0]
            vbc = consts.tile([128, 6], f32, tag="vbc")
            for c0 in range(3):
                for T in range(2):
                    nc.sync.dma_start(
                        vbc[:, c0 * 2 + T : c0 * 2 + T + 1],
                        qkv_b.ap()[
                            2 * C + 384 * T + c0 : 2 * C + 384 * (T + 1) : 3
                        ][:, None],
                    )
            # proj bias replicated across partitions
            pb = consts.tile([128, C], f32, tag="pb")
            nc.sync.dma_start(pb, proj_b.ap()[None, :].to_broadcast([128, C]))

            # ones matrix: denominator matmul replicates the column sums of
            # expT onto all 128 output partitions.
            ones = consts.tile([128, 128], MM, tag="ones")
            if mm_cast:
                nc.vector.memset(ones, 1.0)
            else:
                ones_f = consts.tile([128, 128], f32, tag="ones_f")
                nc.vector.memset(ones_f, 1.0)
                nc.vector.tensor_copy(ones, ones_f)

            def transpose4(srcs, dst):
                """PE-transpose up to four 128x128 chunks into one PSUM tile,
                with one grouped copy to dst (a [128, 128*len] view)."""
                ps = pst.tile([128, NH], MM, tag="tp4")
                for k, s in enumerate(srcs):
                    nc.tensor.transpose(ps[:, k * 128 : (k + 1) * 128], s, ident)
                w = 128 * len(srcs)
                nc.any.tensor_copy(dst, ps[:, 0:w].rearrange("p (c k) -> p c k", k=128))

            def stage_a(b):
                """XT = x[b].T (rows cast to MM dtype before transposing)"""
                XT = pool_xt.tile([128, CB, N], MM, tag="XT")
                for nb2 in range(NB // 2):
                    xrow = pool_row.tile([128, 2, C], f32, tag="row")
                    nc.sync.dma_start(
                        xrow,
                        xs.ap()[b, nb2 * 256 : (nb2 + 1) * 256, :].rearrange(
                            "(two p) c -> p two c", p=128
                        ),
                    )
                    if mm_cast:
                        src2 = pool_row.tile([128, 2, C], MM, tag="row16")
                        nc.any.tensor_copy(src2, xrow)
                    else:
                        src2 = xrow
                    for two in range(2):
                        nb = nb2 * 2 + two
                        src = src2[:, two]
                        nsl = slice(nb * 128, (nb + 1) * 128)
                        transpose4(
                            [src[:, k * 128 : (k + 1) * 128] for k in range(4)],
                            XT[:, 0:4, nsl],
                        )
                        transpose4(
                            [src[:, (4 + k) * 128 : (5 + k) * 128] for k in range(2)],
                            XT[:, 4:6, nsl],
                        )
                return XT

            # batch-0 x rows load before the big weight DMAs so the PE can
            # start transposing immediately; weights stream chunk-wise in the
            # same order stage B consumes them.
            import contextlib
            _loop_n = int(os.environ.get("BLIP_LOOP", "0"))
            _loop_ctx = tc.For_i(0, _loop_n, 1) if _loop_n else contextlib.nullcontext()
            _loop_ctx.__enter__()
            XT_next = stage_a(0)
            w_view = qkv_w.rearrange("(cb p) o -> p cb o", p=128)
            pw_view = proj_w.rearrange("(cb p) o -> p cb o", p=128)
            for cb in range(CB):
                if mm_cast:
                    st = wstage.tile([128, 3 * C], f32, tag="wst")
                    nc.sync.dma_start(st, w_view[:, cb])
                    nc.any.tensor_copy(Wqk[:, cb], st[:, 0 : 2 * C])
                    for c0 in range(3):
                        nc.any.tensor_copy(
                            Wvg[:, cb, c0 * 256 : (c0 + 1) * 256],
                            st[:, 2 * C + c0 : 3 * C : 3],
                        )
                else:
                    nc.sync.dma_start(Wqk[:, cb], w_view[:, cb, 0 : 2 * C])
                    for c0 in range(3):
                        nc.sync.dma_start(
                            Wvg[:, cb, c0 * 256 : (c0 + 1) * 256],
                            w_view[:, cb, 2 * C + c0 : 3 * C : 3],
                        )
            for cb in range(CB):
                if mm_cast:
                    st = wstage.tile([128, 3 * C], f32, tag="wst")
                    nc.sync.dma_start(st[:, 0:C], pw_view[:, cb])
                    nc.any.tensor_copy(PW[:, cb], st[:, 0:C])
                else:
                    nc.sync.dma_start(PW[:, cb], pw_view[:, cb])

            def stage_b(XT):
                """QT, KT (transposed domain, bias fused), Vg (grouped)"""
                QT = pool_qt.tile([128, CB, N], MM, tag="QT")
                KT = pool_kt.tile([128, CB, N], MM, tag="KT")
                for ob in range(2 * CB):
                    dest = QT if ob < CB else KT
                    dcb = ob % CB
                    ps = psmm.tile([128, 2 * NH], f32, tag="mm")
                    for nh in range(2):
                        for cb in range(CB):
                            nc.tensor.matmul(
                                ps[:, nh * NH : (nh + 1) * NH],
                                Wqk[:, cb, ob * 128 : (ob + 1) * 128],
                                XT[:, cb, nh * NH : (nh + 1) * NH],
                                start=(cb == 0),
                                stop=(cb == CB - 1),
                            )
                    nc.any.tensor_scalar(
                        dest[:, dcb, :], ps, qkb[:, ob : ob + 1], None, op0=add
                    )

                Vg = pool_v.tile([128, NB, C], MM, tag="Vg")
                for mb in range(NB):
                    ps = psmm.tile([128, 2 * NH], f32, tag="mm")
                    for c0, cw in ((0, NH), (NH, C - NH)):
                        for cb in range(CB):
                            nc.tensor.matmul(
                                ps[:, c0 : c0 + cw],
                                XT[:, cb, mb * 128 : (mb + 1) * 128],
                                Wvg[:, cb, c0 : c0 + cw],
                                start=(cb == 0),
                                stop=(cb == CB - 1),
                            )
                    nc.any.tensor_copy(Vg[:, mb, :], ps[:, 0:C])
                return QT, KT, Vg

            def stage_cd(QT, KT, Vg):
                """attention per n-half; returns OT3 [128, (c0,T), n] with
                row (c0, T, p) holding OT row c = 3*(128T+p)+c0."""
                OT3 = pool_ot3.tile([128, 6, N], MM, tag="OT3")
                for nh in range(N // NH):
                    nsl = slice(nh * NH, (nh + 1) * NH)
                    expT = pool_expt.tile([128, NB * NH], MM, tag="expT")
                    for mb2 in range(NB // 2):
                        ps = psmm.tile([128, 2 * NH], f32, tag="mm")
                        for two in range(2):
                            for cb in range(CB):
                                nc.tensor.matmul(
                                    ps[:, two * NH : (two + 1) * NH],
                                    KT[:, cb, (mb2 * 2 + two) * 128 : (mb2 * 2 + two + 1) * 128],
                                    QT[:, cb, nsl],
                                    start=(cb == 0),
                                    stop=(cb == CB - 1),
                                )
                        nc.scalar.activation(
                            expT[:, mb2 * 2 * NH : (mb2 + 1) * 2 * NH],
                            ps,
                            mybir.ActivationFunctionType.Exp,
                            scale=SCALE,
                        )

                    dps = psmm.tile([128, 2 * NH], f32, tag="mm")
                    for mb in range(NB):
                        nc.tensor.matmul(
                            dps[:, 0:NH], ones, expT[:, mb * NH : (mb + 1) * NH],
                            start=(mb == 0), stop=(mb == NB - 1),
                        )
                    recipB = pool_rb.tile([128, NH], f32, tag="recipB")
                    nc.vector.reciprocal(recipB, dps[:, 0:NH])
                    recipS = pool_rb.tile([128, NH], MM, tag="recipS")
                    nc.any.tensor_scalar(
                        recipS, recipB, float(ASCALE), None, op0=mult
                    )
                    expTn = pool_exptn.tile([128, NB * NH], MM, tag="expTn")
                    for mb in range(NB):
                        nc.any.tensor_tensor(
                            expTn[:, mb * NH : (mb + 1) * NH],
                            expT[:, mb * NH : (mb + 1) * NH],
                            recipS,
                            op=mult,
                        )

                    for c0 in range(3):
                        ps = psmm.tile([128, 2 * NH], f32, tag="mm")
                        for T in range(2):
                            lo = c0 * 256 + T * 128
                            for mb in range(NB):
                                nc.tensor.matmul(
                                    ps[:, T * NH : (T + 1) * NH],
                                    Vg[:, mb, lo : lo + 128],
                                    expTn[:, mb * NH : (mb + 1) * NH],
                                    start=(mb == 0), stop=(mb == NB - 1),
                                )
                        for T in range(2):
                            nc.any.tensor_scalar(
                                OT3[:, c0 * 2 + T, nsl],
                                ps[:, T * NH : (T + 1) * NH],
                                1.0 / ASCALE,
                                vbc[:, c0 * 2 + T : c0 * 2 + T + 1],
                                op0=mult, op1=add,
                            )
                return OT3

            def stage_e(OT3, b):
                """y = P @ proj_w + proj_b, P = flat(OT) viewed [N, C].
                PT_g[jb][jp, h*128+t'] = PT[jb*128+jp, i], i = 512T+4t'+r,
                h = 2r+T; proj output rows un-permute via stride-4 DMA."""
                PTg = pool_ptg.tile([128, CB, N], MM, tag="PTg")
                for jb in range(CB):
                    for half in range(2):
                        srcs = []
                        for h in range(half * 4, half * 4 + 4):
                            r, T = h // 2, h % 2
                            q = 6 * r + jb
                            c0, nb = q // 8, q % 8
                            srcs.append(
                                OT3[:, c0 * 2 + T, nb * 128 : (nb + 1) * 128]
                            )
                        transpose4(
                            srcs,
                            PTg[:, jb, half * NH : (half + 1) * NH].rearrange(
                                "p (c k) -> p c k", k=128
                            ),
                        )
                for h in range(8):
                    r, T = h // 2, h % 2
                    ps = psmm.tile([128, 2 * NH], f32, tag="mm")
                    for o0, ow in ((0, NH), (NH, C - NH)):
                        for jb in range(CB):
                            nc.tensor.matmul(
                                ps[:, o0 : o0 + ow],
                                PTg[:, jb, h * 128 : (h + 1) * 128],
                                PW[:, jb, o0 : o0 + ow],
                                start=(jb == 0),
                                stop=(jb == CB - 1),
                            )
                    yrow = pool_row.tile([128, C], f32, tag="yrow")
                    nc.any.tensor_tensor(yrow, ps[:, 0:C], pb, op=add)
                    nc.sync.dma_start(
                        y.ap()[b, T * 512 + r : T * 512 + 512 : 4, :], yrow
                    )

            # Software pipeline across batches: next batch's transposes and
            # QKV matmuls are emitted before this batch's projection stage so
            # the scheduler can fill stage-E's stretch with PE work.
            qkv = stage_b(XT_next)
            for b in range(BPC):
                if b + 1 < BPC:
                    XT = stage_a(b + 1)
                OT3 = stage_cd(*qkv)
                if b + 1 < BPC:
                    qkv = stage_b(XT)
                stage_e(OT3, b)
            _loop_ctx.__exit__(None, None, None)

    nc.compile()
    return nc


def _get_nc():
    mm_dtype = os.environ.get("BLIP_MM_DTYPE", "float16")
    key = ("nc", mm_dtype)
    if key not in _CACHE:
        _CACHE[key] = _build(mm_dtype)
    return _CACHE[key]


def kernel(x, qkv_w, qkv_b, proj_w, proj_b, _trace=False, _tmpdir=None):
    x = np.ascontiguousarray(np.asarray(x, dtype=np.float32))
    shared = {
        "qkv_w": np.ascontiguousarray(np.asarray(qkv_w, dtype=np.float32)),
        "qkv_b": np.ascontiguousarray(np.asarray(qkv_b, dtype=np.float32)),
        "proj_w": np.ascontiguousarray(np.asarray(proj_w, dtype=np.float32)),
        "proj_b": np.ascontiguousarray(np.asarray(proj_b, dtype=np.float32)),
    }
    nc = _get_nc()
    in_maps = [
        {"xs": x[c * BPC : (c + 1) * BPC], **shared} for c in range(NCORES)
    ]
    res = run_bass_kernel_spmd(
        nc, in_maps, core_ids=list(range(NCORES)),
        trace=_trace, tmpdir=_tmpdir,
        **({"trace_cores": [0]} if _trace else {}),
    )
    out = np.concatenate([res.results[c]["y"] for c in range(NCORES)], axis=0)
    if _trace:
        return out, res
    return out
